# revision 19
# baseline (speedup 1.0000x reference)
"""Distributed causal attention w/ RoPE for TRN2 (8 NeuronCores).

Sharding: tensor-parallel over heads (2 heads/core). Per core:
  - QKV projection in transposed layout (free dim = 512-token chunks),
    RoPE via pair-swapped duplicates (strided SBUF DMA) + DVE mul/adds.
    Softmax scale (1/sqrt(D)) folded into the exp activation's scale.
  - V transposed to [tk, d] via PE transposes (both heads per 128-block in
    one transpose); each V block augmented with 64 ones-columns so the AV
    matmul emits y (rows 0:64) AND the replicated softmax denominator
    (rows 64:128) in one pass - no separate denominator matmul.
  - Causal attention per (batch, head, 512-token query group), scores
    [tk, tq], column-trimmed to the causal region (c0 = 128*r on diagonal
    blocks); AV accumulates subranges via per-element PSUM has_written.
  - Output projection via chunked AllGather of the per-core head channels
    y^T [128, 512] -> [1024, 512], then a local out-channel-sliced matmul.
    Each core emits out^T channels [128r, 128r+128) per query group.
  - Software-pipelined emission: QKV chunk g+1 is issued before attention
    group g; the pending out-projection of group g-1 is issued after the
    final AV of group g so the AllGather latency is hidden.
Host side: input layout prep (transpose/permute of x and weights) and
concatenation of disjoint output shards.
"""

import numpy as np

import concourse.bass as bass
import concourse.bacc as bacc
import concourse.mybir as mybir
from concourse import tile
from concourse.bass_utils import run_bass_kernel_spmd

B, T, C, H, D = 2, 2048, 1024, 16, 64
NCORE = 8
HPC = H // NCORE          # heads per core = 2
TCH = 512                 # token chunk (qkv proj free dim & query group)
NTC = T // TCH            # 4
NBLK = T // 128           # 16 tk blocks per batch
ROPE_BASE = 10000.0
F32 = mybir.dt.float32
F16 = mybir.dt.float16


def _rope_tables():
    # row p of a q/k tile holds head_local = p // 64, d = p % 64
    d = np.arange(D)
    j = d // 2
    theta = ROPE_BASE ** (-(2.0 * j) / D)          # per-row theta
    t = np.arange(T, dtype=np.float64)
    ang = t[None, :] * theta[:, None]              # [64, T]
    cos = np.cos(ang)
    sin = np.sin(ang)
    sgn = np.where(d % 2 == 0, -1.0, 1.0)[:, None]
    c1 = np.concatenate([cos, cos], axis=0)        # [128, T]
    s1 = np.concatenate([sgn * sin, sgn * sin], axis=0)
    return c1.astype(np.float16), s1.astype(np.float16)


def build(debug=False):
    nc = bacc.Bacc(num_devices=NCORE)
    x_t = nc.declare_dram_parameter("x_t", [B, C, T], F16, isOutput=False)
    w_all = nc.declare_dram_parameter("w_all", [C, 384], F16, isOutput=False)
    w_p = nc.declare_dram_parameter("w_p", [C, 128], F16, isOutput=False)
    out_ext = nc.declare_dram_parameter("out", [B * NTC, 128, TCH], F16,
                                        isOutput=True)
    dbg = {}
    if debug:
        for nm, shp, dt in [("sw_q", [128, TCH], F16), ("ropeq", [128, T], F16),
                            ("ropek", [128, T], F16), ("va0", [128, NBLK, 128], F16),
                            ("e00", [128, TCH], F16), ("rbc00", [64, TCH], F32),
                            ("ypair0", [128, TCH], F16), ("yfull0", [128, 8, TCH], F16)]:
            dbg[nm] = nc.declare_dram_parameter("dbg_" + nm, shp, dt, isOutput=True)

    c1_np, s1_np = _rope_tables()
    ident_np = np.eye(128, dtype=np.float16)
    tk = np.arange(128)[:, None]
    tq = np.arange(128)[None, :]
    mask_np = (tq >= tk).astype(np.float16)

    c1_c = nc.inline_tensor(c1_np, name="c1")
    s1_c = nc.inline_tensor(s1_np, name="s1")
    ident_c = nc.inline_tensor(ident_np, name="ident")
    mask_c = nc.inline_tensor(mask_np, name="mask0")

    NP = B * NTC // 2
    cc_in = [nc.dram_tensor(f"cc_in{j}", [128, 2 * TCH], F16)
             for j in range(NP)]
    cc_out = [nc.dram_tensor(f"cc_out{j}", [C, 2 * TCH], F16,
                             addr_space="Shared") for j in range(NP)]
    kick_in = [nc.inline_tensor(np.zeros((128, 8), np.float16),
                                name=f"kick_in{i}") for i in range(5)]
    kick_out = [nc.dram_tensor(f"kick_out{i}", [C, 8], F16,
                               addr_space="Shared") for i in range(5)]
    groups = [list(range(NCORE))]
    EXPF = mybir.ActivationFunctionType.Exp
    MUL = mybir.AluOpType.mult

    with tile.TileContext(nc) as tc:
        with (
            tc.tile_pool(name="const", bufs=1) as cpool,
            tc.tile_pool(name="big", bufs=2) as bpool,
            tc.tile_pool(name="vaug", bufs=2) as vpool,
            tc.tile_pool(name="tmp", bufs=2) as tpool,
            tc.tile_pool(name="exp", bufs=6) as epool,
            tc.tile_pool(name="norm", bufs=2) as npool,
            tc.tile_pool(name="pqkv", bufs=2, space="PSUM") as qkpool,
            tc.tile_pool(name="psc", bufs=2, space="PSUM") as spool,
            tc.tile_pool(name="py", bufs=1, space="PSUM") as ypool,
            tc.tile_pool(name="pt", bufs=1, space="PSUM") as tppool,
            tc.tile_pool(name="po", bufs=1, space="PSUM") as opool,
        ):
            # ---- persistent SBUF loads; order = DMA priority ----
            ident_sb = cpool.tile([128, 128], F16, tag="ident")
            nc.sync.dma_start(out=ident_sb[:, :], in_=ident_c[:, :])
            w_sb = cpool.tile([128, 8, 384], F16, tag="w")
            nc.sync.dma_start(out=w_sb[:, 0, :], in_=w_all[0:128, :])
            x_sb = {}
            for b in range(B):
                for g in range(NTC):
                    x_sb[(b, g)] = cpool.tile([128, 8, TCH], F16,
                                              name=f"x{b}{g}", tag=f"x{b}{g}")
            nc.sync.dma_start(
                out=x_sb[(0, 0)][:, 0:4, :],
                in_=x_t[0].rearrange("(c p) t -> p c t", p=128)[:, 0:4, 0:TCH])
            nc.sync.dma_start(
                out=x_sb[(0, 0)][:, 4:8, :],
                in_=x_t[0].rearrange("(c p) t -> p c t", p=128)[:, 4:8, 0:TCH])
            for wu in range(32):
                wut = tppool.tile([128, 128], F16, name="wut", tag="tp")
                nc.tensor.transpose(wut[:, :], ident_sb[:, :], ident_sb[:, :])
            nc.sync.dma_start(
                out=w_sb[:, 1:8, :],
                in_=w_all[128:1024, :].rearrange("(c p) m -> p c m", p=128))
            c1_sb = cpool.tile([128, T], F16, tag="c1")
            nc.sync.dma_start(out=c1_sb[:, :], in_=c1_c[:, :])
            s1_sb = cpool.tile([128, T], F16, tag="s1")
            nc.sync.dma_start(out=s1_sb[:, :], in_=s1_c[:, :])
            wp_sb = cpool.tile([128, 8, 128], F16, tag="wp")
            nc.sync.dma_start(out=wp_sb[:, :, :],
                              in_=w_p[:, :].rearrange("(c p) m -> p c m", p=128))
            mask_sb = cpool.tile([128, 128], F16, tag="mask")
            nc.sync.dma_start(out=mask_sb[:, :], in_=mask_c[:, :])
            for b in range(B):
                for g in range(NTC):
                    if (b, g) == (0, 0):
                        continue
                    t0 = g * TCH
                    nc.sync.dma_start(
                        out=x_sb[(b, g)][:, :, :],
                        in_=x_t[b].rearrange("(c p) t -> p c t",
                                             p=128)[:, :, t0:t0 + TCH])

            state = {"pq": []}

            def emit_outproj(P):
                yfull = npool.tile([128, 8, 2 * TCH], F16, tag="yfull")
                cc_v = cc_out[P][:, :].rearrange("(c p) t -> p c t", p=128)
                nc.sync.dma_start(out=yfull[:, :, 0:TCH],
                                  in_=cc_v[:, :, 0:TCH])
                nc.sync.dma_start(out=yfull[:, :, TCH:2 * TCH],
                                  in_=cc_v[:, :, TCH:2 * TCH])
                if debug and P == 0:
                    nc.sync.dma_start(out=dbg["yfull0"][:, :, :],
                                      in_=yfull[:, :, 0:TCH])
                for jj in range(2):
                    o_ps = opool.tile([128, TCH], F32, tag="o")
                    for c in range(8):
                        nc.tensor.matmul(
                            o_ps[:, :], wp_sb[:, c, :],
                            yfull[:, c, jj * TCH:(jj + 1) * TCH],
                            start=(c == 0), stop=(c == 7))
                    outsb = npool.tile([128, TCH], F16, tag="outsb")
                    nc.vector.tensor_copy(outsb[:, :], o_ps[:, :])
                    nc.sync.dma_start(out=out_ext[2 * P + jj, :, :],
                                      in_=outsb[:, :])

            def emit_qkv_chunk(b, g, rope_q, rope_k, vT, va):
                t0 = g * TCH
                xt = x_sb[(b, g)]
                ps = []
                swp = []
                for m in range(3):
                    p = qkpool.tile([128, TCH], F32, tag="pqkv")
                    for c in range(8):
                        nc.tensor.matmul(p[:, :],
                                         w_sb[:, c, m * 128:(m + 1) * 128],
                                         xt[:, c, :],
                                         start=(c == 0), stop=(c == 7))
                    ps.append(p)
                    if m < 2:
                        nat = tpool.tile([128, TCH], F16, tag=f"nat{m}")
                        nc.vector.tensor_copy(nat[:, :], p[:, :])
                        sw = tpool.tile([128, TCH], F16, tag=f"swp{m}")
                        nc.vector.stream_shuffle(sw[:, :], nat[:, :],
                                                 mask=[i ^ 1 for i in range(32)])
                        if debug and b == 0 and g == 0 and m == 0:
                            nc.sync.dma_start(out=dbg["sw_q"][:, :], in_=sw[:, :])
                        swp.append((nat, sw))
                nc.vector.tensor_copy(vT[:, t0:t0 + TCH], ps[2][:, :])
                for (dst, (nat, sw)) in zip((rope_q, rope_k), swp):
                    a = tpool.tile([128, TCH], F16, tag="ra")
                    nc.vector.tensor_mul(a[:, :], nat[:, :],
                                         c1_sb[:, t0:t0 + TCH])
                    bb = tpool.tile([128, TCH], F16, tag="rb")
                    nc.vector.tensor_mul(bb[:, :], sw[:, :],
                                         s1_sb[:, t0:t0 + TCH])
                    nc.vector.tensor_add(dst[:, t0:t0 + TCH], a[:, :], bb[:, :])
                for r in range(4):
                    Tt = 4 * g + r
                    tp = tppool.tile([128, 128], F16, tag="tp")
                    nc.tensor.transpose(tp[:, :], vT[:, Tt * 128:(Tt + 1) * 128],
                                        ident_sb[:, :])
                    for h in range(HPC):
                        nc.vector.tensor_copy(va[h][:, Tt, 64:128],
                                              tp[:, h * 64:(h + 1) * 64])

            def emit_attn(b, g, rope_q, rope_k, va):
                t0 = g * TCH
                ntk = 4 * g + 4
                y_ps = [ypool.tile([128, TCH], F32, name=f"y{h}", tag=f"y{h}")
                        for h in range(HPC)]

                def emit_av(h, Tt, e_sb, c0):
                    nc.tensor.matmul(y_ps[h][:, c0:TCH], va[h][:, Tt, :],
                                     e_sb[:, c0:TCH],
                                     start=(Tt == 0), stop=(Tt == ntk - 1),
                                     skip_group_check=True)

                prev = None
                for Tt in range(ntk):
                    diag = (Tt // 4 == g)
                    r = Tt % 4
                    c0 = 128 * r if diag else 0
                    es = []
                    for h in range(HPC):
                        s_ps = spool.tile([128, TCH], F32, tag="s")
                        nc.tensor.matmul(
                            s_ps[:, c0:TCH],
                            rope_k[h * 64:(h + 1) * 64,
                                   Tt * 128:(Tt + 1) * 128],
                            rope_q[h * 64:(h + 1) * 64, t0 + c0:t0 + TCH],
                            start=True, stop=True)
                        e_sb = epool.tile([128, TCH], F16, tag="e")
                        nc.scalar.activation(e_sb[:, c0:TCH], s_ps[:, c0:TCH],
                                             EXPF, scale=0.125)
                        if diag:
                            nc.vector.tensor_mul(e_sb[:, c0:c0 + 128],
                                                 e_sb[:, c0:c0 + 128],
                                                 mask_sb[:, :])
                        if debug and b == 0 and g == 0 and Tt == 0 and h == 0:
                            nc.sync.dma_start(out=dbg["e00"][:, :], in_=e_sb[:, :])
                        es.append(e_sb)
                    if prev is not None:
                        pTt, pes, pc0 = prev
                        for h in range(HPC):
                            emit_av(h, pTt, pes[h], pc0)
                    prev = (Tt, es, c0)
                pTt, pes, pc0 = prev
                for h in range(HPC):
                    emit_av(h, pTt, pes[h], pc0)

                if g % 2 == 0:
                    state["ypair"] = npool.tile([128, 2 * TCH], F16,
                                                name="ypair", tag="ypair")
                y_pair = state["ypair"]
                cy0 = (g % 2) * TCH
                for h in range(HPC):
                    rbc = npool.tile([64, TCH], F32, tag="rbc")
                    nc.vector.reciprocal_approx_fast(out=rbc[:, :],
                                                     in_=y_ps[h][0:64, :])
                    if debug and b == 0 and g == 0 and h == 0:
                        nc.sync.dma_start(out=dbg["rbc00"][:, :], in_=rbc[:, :])
                    nc.vector.scalar_tensor_tensor(
                        y_pair[h * 64:(h + 1) * 64, cy0:cy0 + TCH],
                        y_ps[h][64:128, :],
                        1.0, rbc[:, :], op0=MUL, op1=MUL)
                if debug and b == 0 and g == 0:
                    nc.sync.dma_start(out=dbg["ypair0"][:, :],
                                      in_=y_pair[:, 0:TCH])
                if g % 2 == 1:
                    P = (b * NTC + g) // 2
                    nc.scalar.dma_start(out=cc_in[P][:, :], in_=y_pair[:, :])
                    nc.gpsimd.collective_compute(
                        "AllGather", mybir.AluOpType.bypass,
                        replica_groups=groups,
                        ins=[cc_in[P].ap().opt()],
                        outs=[cc_out[P].ap().opt()])
                    if P < NP - 1:
                        nc.gpsimd.collective_compute(
                            "AllGather", mybir.AluOpType.bypass,
                            replica_groups=groups,
                            ins=[kick_in[2 + P].ap().opt()],
                            outs=[kick_out[2 + P].ap().opt()])
                    state["pq"].append(P)
                    if len(state["pq"]) >= 2:
                        emit_outproj(state["pq"].pop(0))

            kick_sb = cpool.tile([128, 8], F16, tag="kick")
            nc.scalar.dma_start(out=kick_sb[:, :],
                                in_=x_sb[(B - 1, NTC - 1)][:, 0, 0:8])
            for i in range(2):
                nc.scalar.dma_start(out=kick_in[i][:, :], in_=kick_sb[:, :])
                nc.gpsimd.collective_compute(
                    "AllGather", mybir.AluOpType.bypass,
                    replica_groups=groups,
                    ins=[kick_in[i].ap().opt()],
                    outs=[kick_out[i].ap().opt()])

            for b in range(B):
                rope_q = bpool.tile([128, T], F16, tag="rope_q")
                rope_k = bpool.tile([128, T], F16, tag="rope_k")
                vT = bpool.tile([128, T], F16, tag="vT")
                va = [vpool.tile([128, NBLK, 128], F16, name=f"va{h}",
                                 tag=f"va{h}") for h in range(HPC)]
                for h in range(HPC):
                    nc.gpsimd.memset(va[h][:, :, 0:64], 1.0)
                for g in range(NTC):
                    emit_qkv_chunk(b, g, rope_q, rope_k, vT, va)
                    if g >= 1:
                        emit_attn(b, g - 1, rope_q, rope_k, va)
                if debug and b == 0:
                    nc.sync.dma_start(out=dbg["ropeq"][:, :], in_=rope_q[:, :])
                    nc.sync.dma_start(out=dbg["ropek"][:, :], in_=rope_k[:, :])
                    nc.sync.dma_start(out=dbg["va0"][:, :, :], in_=va[0][:, :, :])
                emit_attn(b, NTC - 1, rope_q, rope_k, va)
            for j in state["pq"]:
                emit_outproj(j)

    if not nc.is_finalized():
        nc.finalize()
    return nc


_NC_CACHE = None


def _get_nc():
    global _NC_CACHE
    if _NC_CACHE is None:
        _NC_CACHE = build()
    return _NC_CACHE


def make_in_maps(x, w_qkv, w_proj):
    x_t = np.ascontiguousarray(
        np.asarray(x, np.float32).transpose(0, 2, 1)).astype(np.float16)
    w_qkv = np.asarray(w_qkv, np.float32)
    w_proj = np.asarray(w_proj, np.float32)
    in_maps = []
    for r in range(NCORE):
        ha, hb = 2 * r, 2 * r + 1
        qrows = (list(range(ha * 64, ha * 64 + 64))
                 + list(range(hb * 64, hb * 64 + 64)))
        rows = (qrows + [1024 + i for i in qrows] + [2048 + i for i in qrows])
        w_all = np.ascontiguousarray(w_qkv[rows, :].T).astype(np.float16)
        w_p = np.ascontiguousarray(
            w_proj[r * 128:(r + 1) * 128, :].T).astype(np.float16)
        in_maps.append({"x_t": x_t, "w_all": w_all, "w_p": w_p})
    return in_maps


def assemble(results):
    outT = np.zeros((B, C, T), np.float32)
    for r in range(NCORE):
        o = results[r]["out"].astype(np.float32)
        for b in range(B):
            for g in range(NTC):
                outT[b, r * 128:(r + 1) * 128, g * TCH:(g + 1) * TCH] = \
                    o[b * NTC + g]
    return np.ascontiguousarray(outT.transpose(0, 2, 1))


def run(x, w_qkv, w_proj, trace=False):
    nc = _get_nc()
    in_maps = make_in_maps(x, w_qkv, w_proj)
    res = run_bass_kernel_spmd(nc, in_maps, list(range(NCORE)), trace=trace)
    return assemble(res.results), res


def kernel(x, w_qkv, w_proj):
    out, _ = run(x, w_qkv, w_proj, trace=False)
    return out
